# revision 1
# baseline (speedup 1.0000x reference)
"""Trainium2 Bass kernel for the differentiable gaussian renderer.

Math: for each batch b, pixel (y,x), channel c:
    out[b,c,y,x] = num/den,
    num = sum_n colors[n,c] * w[n,y,x],  den = eps + sum_n w[n,y,x]
    w[n,y,x] = opac_n * exp(-0.5*((x-u_n)^2 + (y-v_n)^2)/s_n^2)

Key restructuring: w is separable over the pixel grid, so we only need
exp on [N,128] y-factors and [N,32] x-factors (64x fewer exps than the
dense [N,HW] formulation), and num/den become a single accumulating
bf16 matmul over N on the tensor engine:
    psum[y, (k,x)] = sum_n Yexp[n,y] * (s4[n,k] * Xexp[n,x])
where s4 = [opac, opac*c0, opac*c1, opac*c2] (k=0 gives den).

Sharding: 8 cores = 2 batches x 4 pixel-column strips of 32. No
collectives; the host just concatenates the per-core strip outputs.
"""

import math
from contextlib import ExitStack

import numpy as np

import concourse.bacc as bacc
import concourse.bass as bass
import concourse.mybir as mybir
import concourse.tile as tile
from concourse.bass_utils import run_bass_kernel_spmd

f32 = mybir.dt.float32
bf16 = mybir.dt.bfloat16

H, W = 128, 128
FX, FY = 150.0, 150.0
CX, CY = 64.0, 64.0
EPS = 1e-8
N, B = 4096, 2
P = 128          # SBUF partitions; gaussians per contraction chunk
J = N // P       # 32 chunks
NSTRIP = 4       # x strips
SW = W // NSTRIP  # 32 columns per strip
# pipeline segment sizes (chunks): small first segment so ACT starts early,
# small last segment so the post-exp tail (rhs+matmul+divide) is short
SEGS = [3, 8, 8, 8, 4, 1]
assert sum(SEGS) == J
DVE_SQ_SEGS = {4, 5}  # segments whose square runs on the DVE instead of ACT
FUSE_TRAIL = {}  # segment -> #trailing chunks argbuilt+squared fused on ACT
RT = math.sqrt(0.5)

# row-major R layout (so the projection can read rows with affine APs);
# the x2 off-diagonal scale still works on the contiguous triples
# {R01,R02,R10} and {R12,R20,R21}
R00, R01, R02, R10, R11, R12, R20, R21, R22 = range(9)

AF = mybir.ActivationFunctionType
ALU = mybir.AluOpType


def _emit(nc, tc, aps):
    with ExitStack() as ctx:
        pool = ctx.enter_context(tc.tile_pool(name="main", bufs=1))
        args_pool = ctx.enter_context(tc.tile_pool(name="args", bufs=1))
        sq_pool = ctx.enter_context(tc.tile_pool(name="sq", bufs=3))
        exp_pool = ctx.enter_context(tc.tile_pool(name="exp", bufs=1))
        rhs_pool = ctx.enter_context(tc.tile_pool(name="rhs", bufs=3))
        psum_pool = ctx.enter_context(tc.tile_pool(name="psum", bufs=1, space="PSUM"))

        # ---------- load inputs (3 DMAs: packed gaussian data + broadcast
        # smalls, qvec/Mq part first) ----------
        # gdata[p, j, :] = [posx posy posz 1 c0 c1 c2 opac scal] of gaussian
        # j*128+p (host pre-packs, contiguous in DRAM)
        gd = pool.tile([P, J, 9], f32, tag="gd")
        smg = pool.tile([P, 160], f32, tag="smg")  # [ygrid xgrid]
        smq = pool.tile([P, 196], f32, tag="smq")  # [qvec Mq(12x16)]
        QOF, MOF = H + SW, 4
        # qvec+tvec first (tiny, own tile) — the serial R chain depends only
        # on them and must not wait for the grid DMA
        nc.sync.dma_start(smq[:], aps["smalls"][QOF:].partition_broadcast(P))
        # prime the ACT table set (exp_and_others, which also contains
        # Square) while the DVE is still in prep — avoids a ~1.3us
        # PSEUDO_LOAD stall in the middle of the pipeline
        warm = pool.tile([P, 1], f32, tag="warm")
        nc.gpsimd.memset(warm[:], 0.0)
        nc.scalar.activation(warm[:], warm[:], AF.Exp)
        nc.scalar.activation(warm[:], warm[:], AF.Square)
        # constants for the eps-fold matmul at the end of the accumulation
        ones1 = pool.tile([1, P], f32, tag="ones1")
        epsr = pool.tile([1, SW], f32, tag="epsr")
        nc.gpsimd.memset(ones1[:], 1.0)
        nc.gpsimd.memset(epsr[:], EPS)
        nc.gpsimd.dma_start(gd[:], aps["gdata"])  # SWDGE: parallel queue
        nc.sync.dma_start(
            smg[:], aps["smalls"][0:QOF].partition_broadcast(P)
        )
        ygrid = smg[:, 0:H]
        xgrid = smg[:, H:H + SW]

        # ---------- quaternion -> projection coefficients ----------
        # qq[4j+i] = q_i*q_j in one op via repeat-read APs. Every R entry is
        # LINEAR in qq, and so is T'_i = t_i*S (S = |q|^2), so the host ships
        # a per-batch 12x16 matrix Mq with rT = Mq @ qq laid out as
        # [R00 R01 R02 T0' | R10 R11 R12 T1' | R20 R21 R22 T2'].
        # The reference normalizes the quaternion (R/S); S cancels in
        # u = pcx/pcz when t is scaled by S instead — no reciprocal needed.
        smp = smq[:].ap[0]
        qoff = smq[:].offset
        qrep = bass.AP(smq[:].tensor, qoff, [smp, [0, 4], [1, 4]])
        qtrep = bass.AP(smq[:].tensor, qoff, [smp, [1, 4], [0, 4]])
        qq = pool.tile([P, 16], f32, tag="qq")
        nc.vector.tensor_tensor(qq[:], qrep, qtrep, ALU.mult)

        mt = pool.tile([P, 12, 16], f32, tag="mt")
        rT = pool.tile([P, 12], f32, tag="rT")
        qqrep = bass.AP(qq[:].tensor, qq[:].offset, [qq[:].ap[0], [0, 12], [1, 16]])
        nc.vector.tensor_tensor(mt[:], qqrep, smq[:, MOF:MOF + 192], ALU.mult)
        nc.vector.tensor_reduce(rT[:], mt[:], mybir.AxisListType.X, ALU.add)

        # ---------- project gaussians: 2 whole-array ops ----------
        # gd fields: [posx posy posz 1 | c0 c1 c2 | opac scal], so
        # pc[p, i, j] = sum_{c<4} gd[p, j, c] * rT[4i + c]
        opac, scal = gd[:, :, 7], gd[:, :, 8]
        m_ = pool.tile([P, 3, J, 4], f32, tag="m_")
        pc = pool.tile([P, 3, J], f32, tag="pc")
        gpart = gd[:].ap[0]
        posrep = bass.AP(gd[:].tensor, gd[:].offset, [gpart, [0, 3], [9, J], [1, 4]])
        rrep = bass.AP(rT[:].tensor, rT[:].offset,
                       [rT[:].ap[0], [4, 3], [0, J], [1, 4]])
        nc.vector.tensor_tensor(m_[:], posrep, rrep, ALU.mult)
        nc.vector.tensor_reduce(pc[:], m_[:], mybir.AxisListType.X, ALU.add)
        pcx, pcy, pcz = pc[:, 0, :], pc[:, 1, :], pc[:, 2, :]

        invz = pool.tile([P, J], f32, tag="invz")
        nc.vector.reciprocal_approx_fast(invz[:], pcz)
        # The host pre-scales Mq's x/y rows by -F*sqrt(.5) and pre-shifts the
        # grids by -C, so uvt = pc_xy/pcz is already -sqrt(.5)*F*(u - C)/F...
        # i.e. buv = uvt/scale is directly the (negated) argbuild bias.
        uvt = pool.tile([P, 2, J], f32, tag="uvt")
        invzrep = bass.AP(invz[:].tensor, invz[:].offset,
                          [invz[:].ap[0], [0, 2], [1, J]])
        nc.vector.tensor_tensor(uvt[:], pc[:, 0:2, :], invzrep, ALU.mult)

        aa = pool.tile([P, J], f32, tag="aa")     # 1/scale
        buv = pool.tile([P, 2, J], f32, tag="buv")  # [u'/scale, v'/scale]
        nc.vector.reciprocal_approx_fast(aa[:], scal)
        aarep = bass.AP(aa[:].tensor, aa[:].offset, [aa[:].ap[0], [0, 2], [1, J]])
        nc.vector.tensor_tensor(buv[:], uvt[:], aarep, ALU.mult)
        bu, bv = buv[:, 0, :], buv[:, 1, :]

        # s4[:, k, j]: per-gaussian weights [opac, opac*c] — on gpsimd, which
        # is otherwise idle
        s4b = pool.tile([P, 4, J], bf16, tag="s4b")
        nc.gpsimd.tensor_copy(s4b[:, 0, :], opac)
        for k in range(3):
            nc.gpsimd.tensor_tensor(s4b[:, 1 + k, :], opac, gd[:, :, 4 + k], ALU.mult)

        # ---------- pipelined main phase ----------
        # Per segment s: argbuild (DVE) -> square+exp (ACT) -> rhs-build
        # (DVE) -> accumulate matmuls (PE). rhs(s-1) is emitted after
        # argbuild(s) so the in-order DVE stream never waits on ACT.
        acc = psum_pool.tile([P, 4 * SW], f32, tag="acc")
        expv_tiles = []

        def emit_argbuild(s, j0, seg):
            nf = FUSE_TRAIL.get(s, 0)
            args = args_pool.tile([P, seg, H + SW], f32, tag=f"args{s}")
            argsq = sq_pool.tile([P, seg, H + SW], f32, tag="argsq")
            expv = exp_pool.tile([P, seg, H + SW], bf16, tag=f"expv{s}")
            expv_tiles.append(expv)
            for jj in range(seg - nf):
                j = j0 + jj
                nc.vector.tensor_scalar(
                    args[:, jj, 0:H], ygrid, aa[:, j:j + 1], bv[:, j:j + 1],
                    ALU.mult, ALU.add,
                )
                nc.vector.tensor_scalar(
                    args[:, jj, H:H + SW], xgrid, aa[:, j:j + 1], bu[:, j:j + 1],
                    ALU.mult, ALU.add,
                )
            for jj in range(seg - nf, seg):
                # trailing chunks: argbuild FUSED into the square on ACT
                # (Square(in*scale + bias)) to offload the DVE spine
                j = j0 + jj
                nc.scalar.activation(
                    argsq[:, jj, 0:H], ygrid, AF.Square,
                    bias=bv[:, j:j + 1], scale=aa[:, j:j + 1],
                )
                nc.scalar.activation(
                    argsq[:, jj, H:H + SW], xgrid, AF.Square,
                    bias=bu[:, j:j + 1], scale=aa[:, j:j + 1],
                )
            if seg - nf > 0:
                if s in DVE_SQ_SEGS:
                    # square on the DVE — it is otherwise idle at this point
                    nc.vector.tensor_tensor(
                        argsq[:, 0:seg - nf, :], args[:, 0:seg - nf, :],
                        args[:, 0:seg - nf, :], ALU.mult,
                    )
                else:
                    nc.scalar.activation(
                        argsq[:, 0:seg - nf, :], args[:, 0:seg - nf, :], AF.Square
                    )
            nc.scalar.activation(expv[:], argsq[:], AF.Exp, scale=-1.0)

        def emit_consume(s, j0, seg):
            # rhs[:, jj, k*SW + x] = xexp[jj, x] * s4b[k, j0+jj] for the whole
            # segment in ONE op via repeat-read APs (step-0 dims)
            expv = expv_tiles[s]
            rhs = rhs_pool.tile([P, seg, 4 * SW], bf16, tag="rhs")
            part = expv[:].ap[0]
            xrep = bass.AP(
                expv[:].tensor, expv[:, 0, H:H + SW].offset,
                [part, [H + SW, seg], [0, 4], [1, SW]],
            )
            srep = bass.AP(
                s4b[:].tensor, s4b[:, :, j0].offset,
                [s4b[:].ap[0], [1, seg], [J, 4], [0, SW]],
            )
            rout = bass.AP(
                rhs[:].tensor, rhs[:].offset,
                [rhs[:].ap[0], [4 * SW, seg], [SW, 4], [1, SW]],
            )
            nc.vector.tensor_tensor(rout, xrep, srep, ALU.mult)
            for jj in range(seg):
                j = j0 + jj
                nc.tensor.matmul(
                    acc[:], expv[:, jj, 0:H], rhs[:, jj, :],
                    start=(j == 0), stop=False,
                )

        starts = [sum(SEGS[:i]) for i in range(len(SEGS))]
        for s, (j0, seg) in enumerate(zip(starts, SEGS)):
            emit_argbuild(s, j0, seg)
            if s >= 1:
                emit_consume(s - 1, starts[s - 1], SEGS[s - 1])
        emit_consume(len(SEGS) - 1, starts[-1], SEGS[-1])
        # fold den += EPS into the accumulation group (K=1 matmul of
        # ones x eps) so the divide chain needs no separate add
        nc.tensor.matmul(acc[:, 0:SW], ones1[:], epsr[:], start=False, stop=True)

        # ---------- divide and store ----------
        dinv = pool.tile([P, SW], f32, tag="dinv")
        nc.vector.reciprocal_approx_fast(dinv[:], acc[:, 0:SW])
        outsb = pool.tile([P, 3, SW], f32, tag="outsb")
        dinv3 = bass.AP(
            dinv[:].tensor, dinv[:].offset, [dinv[:].ap[0], [0, 3], [1, SW]]
        )
        nc.vector.tensor_tensor(outsb[:], acc[:, SW:4 * SW], dinv3, ALU.mult)
        nc.sync.dma_start(aps["out"].rearrange("c y x -> y c x"), outsb[:])


def build_nc(num_devices=8, debug=False):
    nc = bacc.Bacc(
        "TRN2", target_bir_lowering=False, debug=debug, num_devices=num_devices
    )
    aps = {
        "gdata": nc.dram_tensor("gdata", [P, J, 9], f32, kind="ExternalInput").ap(),
        "smalls": nc.dram_tensor("smalls", [356], f32, kind="ExternalInput").ap(),
        "out": nc.dram_tensor("out", [3, H, SW], f32, kind="ExternalOutput").ap(),
    }
    with tile.TileContext(nc) as tc:
        _emit(nc, tc, aps)
    nc.compile()
    return nc


def _build_mq():
    """[R00..R22 row-major, S] = Mq @ qq, where qq[4j+i] = q_i*q_j."""
    mq = np.zeros((10, 16), np.float32)

    def f(i, j):
        return 4 * j + i

    for row, signs in [(R00, (1, 1, -1, -1)), (R11, (1, -1, 1, -1)),
                       (R22, (1, -1, -1, 1)), (9, (1, 1, 1, 1))]:
        for k, sgn in enumerate(signs):
            mq[row, f(k, k)] = sgn
    # off-diagonals: 2*(q_a q_b + sgn * q_c q_d), using both symmetric slots
    for row, (a, b), (c, d), sgn in [
        (R01, (1, 2), (3, 0), -1),
        (R02, (1, 3), (2, 0), +1),
        (R10, (1, 2), (3, 0), +1),
        (R12, (2, 3), (1, 0), -1),
        (R20, (1, 3), (2, 0), -1),
        (R21, (2, 3), (1, 0), +1),
    ]:
        mq[row, f(a, b)] = mq[row, f(b, a)] = 1.0
        mq[row, f(c, d)] = mq[row, f(d, c)] = sgn
    return mq


def make_in_maps(positions, colors, opacities, scales, qvec, tvec):
    # pack gaussian data as gdata[p, j, :] = [pos3 1 col3 opac scal] of
    # gaussian j*128+p, contiguous for a single fast DMA (the constant 1
    # lets the projection's length-4 reduce include the translation)
    gdata = np.empty((N, 9), np.float32)
    gdata[:, 0:3] = np.asarray(positions, np.float32)
    gdata[:, 3] = 1.0
    gdata[:, 4:7] = np.asarray(colors, np.float32)
    gdata[:, 7] = np.asarray(opacities, np.float32).reshape(N)
    gdata[:, 8] = np.asarray(scales, np.float32).reshape(N)
    gdata = np.ascontiguousarray(
        gdata.reshape(J, P, 9).transpose(1, 0, 2)
    )
    # grids pre-scaled by sqrt(.5) and pre-shifted by the principal point;
    # Mq's x/y rows carry -F*sqrt(.5) so that the device's uvt = pc_xy/pcz
    # is directly the negated argbuild bias (times 1/scale)
    ygrid = ((np.arange(H, dtype=np.float32) - CY) * RT).astype(np.float32)
    qvec = np.asarray(qvec, np.float32)
    tvec = np.asarray(tvec, np.float32)
    mq = _build_mq()
    srow = mq[9]
    in_maps = []
    for core in range(8):
        b, strip = core // NSTRIP, core % NSTRIP
        xgrid = ((np.arange(SW, dtype=np.float32) + strip * SW - CX) * RT
                 ).astype(np.float32)
        # per-batch 12x16: [R-row_i | t_i * S-row] interleaved, camera rows
        # (i=0,1) scaled by -F*sqrt(.5)
        mq12 = np.empty((12, 16), np.float32)
        for i in range(3):
            rowscale = -FX * RT if i < 2 else 1.0
            mq12[4 * i:4 * i + 3] = mq[3 * i:3 * i + 3] * rowscale
            mq12[4 * i + 3] = tvec[b][i] * srow * rowscale
        smalls = np.concatenate(
            [ygrid, xgrid, qvec[b], mq12.reshape(-1)]
        ).astype(np.float32)
        in_maps.append({"gdata": gdata, "smalls": smalls})
    return in_maps


_NC_CACHE = {}


def _get_nc():
    if "nc" not in _NC_CACHE:
        _NC_CACHE["nc"] = build_nc()
    return _NC_CACHE["nc"]


def run_spmd(inputs, trace=False, **kw):
    nc = _get_nc()
    in_maps = make_in_maps(**inputs)
    try:
        res = run_bass_kernel_spmd(nc, in_maps, list(range(8)), trace=trace, **kw)
    except Exception:
        # transient device errors (e.g. NRT_EXEC_UNIT_UNRECOVERABLE) have
        # been observed to clear on retry
        res = run_bass_kernel_spmd(nc, in_maps, list(range(8)), trace=trace, **kw)
    out = np.empty((B, 3, H, W), np.float32)
    for core in range(8):
        b, strip = core // NSTRIP, core % NSTRIP
        out[b, :, :, strip * SW:(strip + 1) * SW] = res.results[core]["out"]
    return out, res


def kernel(positions, colors, opacities, scales, qvec, tvec):
    out, _ = run_spmd(dict(
        positions=positions, colors=colors, opacities=opacities,
        scales=scales, qvec=qvec, tvec=tvec,
    ))
    return out



# revision 2
# speedup vs baseline: 1.2245x; 1.2245x over previous
"""Trainium2 Bass kernel for the differentiable gaussian renderer.

Math: for each batch b, pixel (y,x), channel c:
    out[b,c,y,x] = num/den,
    num = sum_n colors[n,c] * w[n,y,x],  den = eps + sum_n w[n,y,x]
    w[n,y,x] = opac_n * exp(-0.5*((x-u_n)^2 + (y-v_n)^2)/s_n^2)

Key restructurings:
  * w is separable over the pixel grid, so we only need exp on [N,128]
    y-factors and [N,32] x-factors (64x fewer exps than the dense [N,HW]
    formulation), and num/den become a single accumulating bf16 matmul
    over N on the tensor engine:
        psum[y, (k,x)] = sum_n Yexp[n,y] * (s4[n,k] * Xexp[n,x])
    where s4 = [opac, opac*c0, opac*c1, opac*c2] (k=0 gives den).
  * tile culling (standard splatting): a gaussian whose nearest pixel in
    a core's strip is further than sqrt(300)*scale contributes
    exp(-0.5*d2/var) with 0.5*d2/var > 150, which underflows to an exact
    float32 zero — dropping it leaves the f32 result bit-unchanged. Each
    core receives only the gaussians that can touch its strip; the device
    program is compiled for the runtime maximum chunk count J.

Sharding: 8 cores = 2 batches x 4 pixel-column strips of 32. No
collectives; the host just concatenates the per-core strip outputs.
"""

import math
from contextlib import ExitStack

import numpy as np

import concourse.bacc as bacc
import concourse.bass as bass
import concourse.mybir as mybir
import concourse.tile as tile
from concourse.bass_utils import run_bass_kernel_spmd

f32 = mybir.dt.float32
bf16 = mybir.dt.bfloat16

H, W = 128, 128
FX, FY = 150.0, 150.0
CX, CY = 64.0, 64.0
EPS = 1e-8
N, B = 4096, 2
P = 128          # SBUF partitions; gaussians per contraction chunk
NSTRIP = 4       # x strips
SW = W // NSTRIP  # 32 columns per strip
RT = math.sqrt(0.5)
# cull threshold: drop when d2 > CULL_THRESH * var, i.e. 0.5*d2/var > 150;
# exp(-150) = 7e-66 is far below the smallest f32 denormal, so dropped
# gaussians contribute exact zeros in the reference's own f32 arithmetic.
CULL_THRESH = 300.0

AF = mybir.ActivationFunctionType
ALU = mybir.AluOpType

GOF = {"posx": 0, "posy": 1, "posz": 2, "one": 3, "c0": 4, "c1": 5, "c2": 6,
       "opac": 7, "scal": 8}


def _segs_for(J):
    """Pipeline segment sizes: small first segment so ACT starts early,
    size-1 last segment so the post-exp tail is short."""
    if J <= 2:
        return [J]
    segs = [min(3, J - 1)]
    rem = J - segs[0] - 1
    while rem > 0:
        take = min(8, rem)
        segs.append(take)
        rem -= take
    segs.append(1)
    return segs


def _emit(nc, tc, aps, J):
    SEGS = _segs_for(J)
    # segments whose square runs on the DVE instead of ACT (late segments,
    # when the DVE's own argbuild backlog has drained)
    nseg = len(SEGS)
    DVE_SQ_SEGS = {nseg - 2, nseg - 1} if J >= 16 else set()

    with ExitStack() as ctx:
        pool = ctx.enter_context(tc.tile_pool(name="main", bufs=1))
        args_pool = ctx.enter_context(tc.tile_pool(name="args", bufs=1))
        sq_pool = ctx.enter_context(tc.tile_pool(name="sq", bufs=3))
        exp_pool = ctx.enter_context(tc.tile_pool(name="exp", bufs=1))
        rhs_pool = ctx.enter_context(tc.tile_pool(name="rhs", bufs=3))
        psum_pool = ctx.enter_context(tc.tile_pool(name="psum", bufs=1, space="PSUM"))

        # ---------- load inputs ----------
        # gdata[p, j, :] = [posx posy posz 1 c0 c1 c2 opac scal] of the
        # p-th gaussian of chunk j (host pre-packs after culling).
        # gdata goes FIRST on the SP/HWDGE queue: the serial projection
        # chain depends on it. smalls ([ygrid xgrid rT]) rides the gpsimd
        # SWDGE queue in parallel.
        gd = pool.tile([P, J, 9], f32, tag="gd")
        sm = pool.tile([P, H + SW + 12], f32, tag="sm")
        nc.sync.dma_start(gd[:], aps["gdata"])
        nc.gpsimd.dma_start(sm[:], aps["smalls"].partition_broadcast(P))
        # prime the ACT table set (exp_and_others, which also contains
        # Square) while waiting on the DMAs
        warm = pool.tile([P, 1], f32, tag="warm")
        nc.gpsimd.memset(warm[:], 0.0)
        nc.scalar.activation(warm[:], warm[:], AF.Exp)
        nc.scalar.activation(warm[:], warm[:], AF.Square)
        # constants for the eps-fold matmul at the end of the accumulation
        ones1 = pool.tile([1, P], f32, tag="ones1")
        epsr = pool.tile([1, SW], f32, tag="epsr")
        nc.gpsimd.memset(ones1[:], 1.0)
        nc.gpsimd.memset(epsr[:], EPS)
        ygrid = sm[:, 0:H]
        xgrid = sm[:, H:H + SW]
        rT = sm[:, H + SW:H + SW + 12]  # [-FRT*R0|-FRT*t0|-FRT*R1|-FRT*t1|R2|t2]

        # ---------- project gaussians: 2 whole-array ops ----------
        # gd fields: [posx posy posz 1 | c0 c1 c2 | opac scal], so
        # pc[p, i, j] = sum_{c<4} gd[p, j, c] * rT[4i + c]
        opac, scal = gd[:, :, 7], gd[:, :, 8]
        m_ = pool.tile([P, 3, J, 4], f32, tag="m_")
        pc = pool.tile([P, 3, J], f32, tag="pc")
        gpart = gd[:].ap[0]
        posrep = bass.AP(gd[:].tensor, gd[:].offset, [gpart, [0, 3], [9, J], [1, 4]])
        rrep = bass.AP(rT.tensor, rT.offset, [rT.ap[0], [4, 3], [0, J], [1, 4]])
        nc.vector.tensor_tensor(m_[:], posrep, rrep, ALU.mult)
        nc.vector.tensor_reduce(pc[:], m_[:], mybir.AxisListType.X, ALU.add)

        invz = pool.tile([P, J], f32, tag="invz")
        nc.vector.reciprocal_approx_fast(invz[:], pc[:, 2, :])
        # rT's x/y rows are pre-scaled by -F*sqrt(.5) and the grids are
        # pre-shifted by -C, so uvt = pc_xy/pcz is directly the negated
        # argbuild bias (times 1/scale).
        uvt = pool.tile([P, 2, J], f32, tag="uvt")
        invzrep = bass.AP(invz[:].tensor, invz[:].offset,
                          [invz[:].ap[0], [0, 2], [1, J]])
        nc.vector.tensor_tensor(uvt[:], pc[:, 0:2, :], invzrep, ALU.mult)

        aa = pool.tile([P, J], f32, tag="aa")     # 1/scale
        buv = pool.tile([P, 2, J], f32, tag="buv")  # [u'/scale, v'/scale]
        nc.vector.reciprocal_approx_fast(aa[:], scal)
        aarep = bass.AP(aa[:].tensor, aa[:].offset, [aa[:].ap[0], [0, 2], [1, J]])
        nc.vector.tensor_tensor(buv[:], uvt[:], aarep, ALU.mult)
        bu, bv = buv[:, 0, :], buv[:, 1, :]

        # s4[:, k, j]: per-gaussian weights [opac, opac*c] — on gpsimd,
        # which is otherwise idle
        s4b = pool.tile([P, 4, J], bf16, tag="s4b")
        nc.gpsimd.tensor_copy(s4b[:, 0, :], opac)
        for k in range(3):
            nc.gpsimd.tensor_tensor(s4b[:, 1 + k, :], opac, gd[:, :, 4 + k], ALU.mult)

        # ---------- pipelined main phase ----------
        # Per segment s: argbuild (DVE) -> square+exp (ACT) -> rhs-build
        # (DVE) -> accumulate matmuls (PE). rhs(s-1) is emitted after
        # argbuild(s) so the in-order DVE stream never waits on ACT.
        acc = psum_pool.tile([P, 4 * SW], f32, tag="acc")
        expv_tiles = []

        def emit_argbuild(s, j0, seg):
            args = args_pool.tile([P, seg, H + SW], f32, tag=f"args{s}")
            argsq = sq_pool.tile([P, seg, H + SW], f32, tag="argsq")
            expv = exp_pool.tile([P, seg, H + SW], bf16, tag=f"expv{s}")
            expv_tiles.append(expv)
            for jj in range(seg):
                j = j0 + jj
                nc.vector.tensor_scalar(
                    args[:, jj, 0:H], ygrid, aa[:, j:j + 1], bv[:, j:j + 1],
                    ALU.mult, ALU.add,
                )
                nc.vector.tensor_scalar(
                    args[:, jj, H:H + SW], xgrid, aa[:, j:j + 1], bu[:, j:j + 1],
                    ALU.mult, ALU.add,
                )
            if s in DVE_SQ_SEGS:
                # square on the DVE — it is otherwise idle at this point
                nc.vector.tensor_tensor(
                    argsq[:], args[:], args[:], ALU.mult,
                )
            else:
                nc.scalar.activation(argsq[:], args[:], AF.Square)
            nc.scalar.activation(expv[:], argsq[:], AF.Exp, scale=-1.0)

        def emit_consume(s, j0, seg):
            # rhs[:, jj, k*SW + x] = xexp[jj, x] * s4b[k, j0+jj] for the
            # whole segment in ONE op via repeat-read APs (step-0 dims)
            expv = expv_tiles[s]
            rhs = rhs_pool.tile([P, seg, 4 * SW], bf16, tag="rhs")
            part = expv[:].ap[0]
            xrep = bass.AP(
                expv[:].tensor, expv[:, 0, H:H + SW].offset,
                [part, [H + SW, seg], [0, 4], [1, SW]],
            )
            srep = bass.AP(
                s4b[:].tensor, s4b[:, :, j0].offset,
                [s4b[:].ap[0], [1, seg], [J, 4], [0, SW]],
            )
            rout = bass.AP(
                rhs[:].tensor, rhs[:].offset,
                [rhs[:].ap[0], [4 * SW, seg], [SW, 4], [1, SW]],
            )
            nc.vector.tensor_tensor(rout, xrep, srep, ALU.mult)
            for jj in range(seg):
                j = j0 + jj
                nc.tensor.matmul(
                    acc[:], expv[:, jj, 0:H], rhs[:, jj, :],
                    start=(j == 0), stop=False,
                )

        starts = [sum(SEGS[:i]) for i in range(len(SEGS))]
        for s, (j0, seg) in enumerate(zip(starts, SEGS)):
            emit_argbuild(s, j0, seg)
            if s >= 1:
                emit_consume(s - 1, starts[s - 1], SEGS[s - 1])
        emit_consume(len(SEGS) - 1, starts[-1], SEGS[-1])
        # fold den += EPS into the accumulation group (K=1 matmul of
        # ones x eps) so the divide chain needs no separate add
        nc.tensor.matmul(acc[:, 0:SW], ones1[:], epsr[:], start=False, stop=True)

        # ---------- divide and store ----------
        dinv = pool.tile([P, SW], f32, tag="dinv")
        nc.vector.reciprocal_approx_fast(dinv[:], acc[:, 0:SW])
        outsb = pool.tile([P, 3, SW], f32, tag="outsb")
        dinv3 = bass.AP(
            dinv[:].tensor, dinv[:].offset, [dinv[:].ap[0], [0, 3], [1, SW]]
        )
        nc.vector.tensor_tensor(outsb[:], acc[:, SW:4 * SW], dinv3, ALU.mult)
        nc.sync.dma_start(aps["out"], outsb[:])


def _emit_empty(nc, tc, aps):
    """J == 0: the culled work list is empty, i.e. no gaussian reaches any
    pixel of this core's strip — every weight is an exact f32 zero, so
    num = 0, den = EPS and the strip renders to exact zeros."""
    with ExitStack() as ctx:
        pool = ctx.enter_context(tc.tile_pool(name="main", bufs=1))
        outsb = pool.tile([P, 3, SW], f32, tag="outsb")
        nc.gpsimd.memset(outsb[:], 0.0)
        nc.sync.dma_start(aps["out"], outsb[:])


def build_nc(J, num_devices=8, debug=False):
    nc = bacc.Bacc(
        "TRN2", target_bir_lowering=False, debug=debug, num_devices=num_devices
    )
    # out[y, c, x]: partition-major like the SBUF result tile; the host
    # transposes each strip back to [c, y, x]
    aps = {"out": nc.dram_tensor("out", [H, 3, SW], f32, kind="ExternalOutput").ap()}
    if J > 0:
        aps["gdata"] = nc.dram_tensor(
            "gdata", [P, J, 9], f32, kind="ExternalInput").ap()
        aps["smalls"] = nc.dram_tensor(
            "smalls", [H + SW + 12], f32, kind="ExternalInput").ap()
    with tile.TileContext(nc) as tc:
        if J > 0:
            _emit(nc, tc, aps, J)
        else:
            _emit_empty(nc, tc, aps)
    nc.compile()
    return nc


def _rot_from_quat(q):
    q = np.asarray(q, np.float64)
    q = q / np.linalg.norm(q)
    w, x, y, z = q
    return np.array([
        [1 - 2 * (y * y + z * z), 2 * (x * y - z * w), 2 * (x * z + y * w)],
        [2 * (x * y + z * w), 1 - 2 * (x * x + z * z), 2 * (y * z - x * w)],
        [2 * (x * z - y * w), 2 * (y * z + x * w), 1 - 2 * (x * x + y * y)],
    ])


def plan(positions, colors, opacities, scales, qvec, tvec):
    """Cull per (batch, strip), pack per-core inputs, choose J."""
    pos = np.asarray(positions, np.float64)
    scal = np.asarray(scales, np.float64).reshape(-1)
    var = scal * scal
    keeps = []
    for b in range(B):
        R = _rot_from_quat(qvec[b])
        p_cam = pos @ R.T + np.asarray(tvec[b], np.float64)
        with np.errstate(divide="ignore", invalid="ignore"):
            u = p_cam[:, 0] / p_cam[:, 2] * FX + CX
            v = p_cam[:, 1] / p_cam[:, 2] * FY + CY
        dy = np.maximum(np.maximum(0.0 - v, v - (H - 1)), 0.0)
        for strip in range(NSTRIP):
            x0, x1 = strip * SW, strip * SW + SW - 1
            dx = np.maximum(np.maximum(x0 - u, u - x1), 0.0)
            d2 = dx * dx + dy * dy
            keeps.append(np.nonzero(d2 <= CULL_THRESH * var)[0])
    J = (max(len(k) for k in keeps) + P - 1) // P
    return keeps, J


def make_in_maps(keeps, J, positions, colors, opacities, scales, qvec, tvec):
    if J == 0:
        return [{} for _ in range(8)]
    # grids pre-scaled by sqrt(.5) and pre-shifted by the principal point;
    # rT's x/y rows carry -F*sqrt(.5) so that the device's uvt = pc_xy/pcz
    # is directly the negated argbuild bias (times 1/scale)
    ygrid = ((np.arange(H, dtype=np.float64) - CY) * RT).astype(np.float32)
    gall = np.empty((N, 9), np.float32)
    gall[:, 0:3] = np.asarray(positions, np.float32)
    gall[:, 3] = 1.0
    gall[:, 4:7] = np.asarray(colors, np.float32)
    gall[:, 7] = np.asarray(opacities, np.float32).reshape(N)
    gall[:, 8] = np.asarray(scales, np.float32).reshape(N)
    # padding gaussian: far in front of the camera, zero opacity
    pad = np.array([0, 0, 1, 1, 0, 0, 0, 0, 1], np.float32)
    in_maps = []
    for core in range(8):
        b, strip = core // NSTRIP, core % NSTRIP
        keep = keeps[core]
        g = np.empty((J * P, 9), np.float32)
        g[:len(keep)] = gall[keep]
        g[len(keep):] = pad
        # gdata[p, j, :] = gaussian j*P + p... laid out [P, J, 9]
        gdata = np.ascontiguousarray(g.reshape(J, P, 9).transpose(1, 0, 2))
        xgrid = ((np.arange(SW, dtype=np.float64) + strip * SW - CX) * RT
                 ).astype(np.float32)
        R = _rot_from_quat(qvec[b])
        t = np.asarray(tvec[b], np.float64)
        rT = np.empty((3, 4), np.float64)
        rT[:, 0:3] = R
        rT[:, 3] = t
        rT[0:2] *= -FX * RT
        smalls = np.concatenate([ygrid, xgrid, rT.reshape(-1).astype(np.float32)])
        in_maps.append({"gdata": gdata, "smalls": smalls.astype(np.float32)})
    return in_maps


_NC_CACHE = {}


def _get_nc(J):
    if J not in _NC_CACHE:
        _NC_CACHE[J] = build_nc(J)
    return _NC_CACHE[J]


def unshard(results):
    """results: per-core dict with 'out' [H, 3, SW] -> full [B, 3, H, W]."""
    out = np.empty((B, 3, H, W), np.float32)
    for core in range(8):
        b, strip = core // NSTRIP, core % NSTRIP
        out[b, :, :, strip * SW:(strip + 1) * SW] = (
            np.asarray(results[core]["out"]).transpose(1, 0, 2)
        )
    return out


def run_spmd(inputs, trace=False, **kw):
    keeps, J = plan(**inputs)
    nc = _get_nc(J)
    in_maps = make_in_maps(keeps, J, **inputs)
    try:
        res = run_bass_kernel_spmd(nc, in_maps, list(range(8)), trace=trace, **kw)
    except Exception:
        # transient device errors (e.g. NRT_EXEC_UNIT_UNRECOVERABLE) have
        # been observed to clear on retry
        res = run_bass_kernel_spmd(nc, in_maps, list(range(8)), trace=trace, **kw)
    return unshard(res.results), res


def kernel(positions, colors, opacities, scales, qvec, tvec):
    out, _ = run_spmd(dict(
        positions=positions, colors=colors, opacities=opacities,
        scales=scales, qvec=qvec, tvec=tvec,
    ))
    return out


# revision 3
# speedup vs baseline: 1.2816x; 1.0466x over previous
"""Trainium2 Bass kernel for the differentiable gaussian renderer.

Math: for each batch b, pixel (y,x), channel c:
    out[b,c,y,x] = num/den,
    num = sum_n colors[n,c] * w[n,y,x],  den = eps + sum_n w[n,y,x]
    w[n,y,x] = opac_n * exp(-0.5*((x-u_n)^2 + (y-v_n)^2)/s_n^2)

Key restructurings:
  * w is separable over the pixel grid, so we only need exp on [N,128]
    y-factors and [N,32] x-factors (64x fewer exps than the dense [N,HW]
    formulation), and num/den become a single accumulating bf16 matmul
    over N on the tensor engine:
        psum[y, (k,x)] = sum_n Yexp[n,y] * (s4[n,k] * Xexp[n,x])
    where s4 = [opac, opac*c0, opac*c1, opac*c2] (k=0 gives den).
  * tile culling (standard splatting): a gaussian whose nearest pixel in
    a core's strip is further than sqrt(300)*scale contributes
    exp(-0.5*d2/var) with 0.5*d2/var > 150, which underflows to an exact
    float32 zero — dropping it leaves the f32 result bit-unchanged. Each
    core receives only the gaussians that can touch its strip; the device
    program is compiled for the runtime maximum chunk count J.

Sharding: 8 cores = 2 batches x 4 pixel-column strips of 32. No
collectives; the host just concatenates the per-core strip outputs.
"""

import math
from contextlib import ExitStack

import numpy as np

import concourse.bacc as bacc
import concourse.bass as bass
import concourse.mybir as mybir
import concourse.tile as tile
from concourse.bass_utils import run_bass_kernel_spmd

f32 = mybir.dt.float32
bf16 = mybir.dt.bfloat16

H, W = 128, 128
FX, FY = 150.0, 150.0
CX, CY = 64.0, 64.0
EPS = 1e-8
N, B = 4096, 2
P = 128          # SBUF partitions; gaussians per contraction chunk
NSTRIP = 4       # x strips
SW = W // NSTRIP  # 32 columns per strip
RT = math.sqrt(0.5)
# cull threshold: drop when d2 > CULL_THRESH * var, i.e. 0.5*d2/var > 150;
# exp(-150) = 7e-66 is far below the smallest f32 denormal, so dropped
# gaussians contribute exact zeros in the reference's own f32 arithmetic.
CULL_THRESH = 300.0

AF = mybir.ActivationFunctionType
ALU = mybir.AluOpType


def _segs_for(J):
    """Pipeline segment sizes: small first segment so ACT starts early,
    size-1 last segment so the post-exp tail is short."""
    if J <= 2:
        return [J]
    segs = [min(3, J - 1)]
    rem = J - segs[0] - 1
    while rem > 0:
        take = min(8, rem)
        segs.append(take)
        rem -= take
    segs.append(1)
    return segs


def _emit(nc, tc, aps, J):
    SEGS = _segs_for(J)
    nseg = len(SEGS)
    # segments whose square runs on the DVE instead of ACT (late segments,
    # when the DVE's own argbuild backlog has drained)
    DVE_SQ_SEGS = {nseg - 2, nseg - 1} if J >= 16 else set()
    GW = J * 9  # flat width of the per-gaussian block in the packed tile

    with ExitStack() as ctx:
        pool = ctx.enter_context(tc.tile_pool(name="main", bufs=1))
        args_pool = ctx.enter_context(tc.tile_pool(name="args", bufs=1))
        sq_pool = ctx.enter_context(tc.tile_pool(name="sq", bufs=3))
        exp_pool = ctx.enter_context(tc.tile_pool(name="exp", bufs=1))
        rhs_pool = ctx.enter_context(tc.tile_pool(name="rhs", bufs=3))
        psum_pool = ctx.enter_context(tc.tile_pool(name="psum", bufs=1, space="PSUM"))

        # ---------- load inputs ----------
        # gdata[p, j*9:(j+1)*9] = [posx posy posz 1 c0 c1 c2 opac scal] of
        # the p-th gaussian of chunk j (host pre-packs after culling), and
        # gdata[p, J*9:J*9+12] = rT (host-replicated): the whole serial
        # projection chain depends on this single DMA, which goes FIRST on
        # the SP/HWDGE queue. The pixel grids ride the gpsimd SWDGE queue
        # in parallel — they are only needed later, at argbuild.
        gd = pool.tile([P, GW + 12], f32, tag="gd")
        sm = pool.tile([P, H + SW], f32, tag="sm")
        nc.sync.dma_start(gd[:], aps["gdata"])
        nc.gpsimd.dma_start(sm[:], aps["smalls"].partition_broadcast(P))
        # prime the ACT table set (exp_and_others, which also contains
        # Square) while waiting on the DMAs
        warm = pool.tile([P, 1], f32, tag="warm")
        nc.gpsimd.memset(warm[:], 0.0)
        nc.scalar.activation(warm[:], warm[:], AF.Exp)
        nc.scalar.activation(warm[:], warm[:], AF.Square)
        # constants for the eps-fold matmul at the end of the accumulation
        ones1 = pool.tile([1, P], f32, tag="ones1")
        epsr = pool.tile([1, SW], f32, tag="epsr")
        nc.gpsimd.memset(ones1[:], 1.0)
        nc.gpsimd.memset(epsr[:], EPS)
        ygrid = sm[:, 0:H]
        xgrid = sm[:, H:H + SW]

        gt, go = gd[:].tensor, gd[:].offset
        gpart = gd[:].ap[0]

        def gfield(c):  # [P, J] view of per-gaussian field c
            return bass.AP(gt, go + c, [gpart, [9, J]])

        opac, scal = gfield(7), gfield(8)
        # rT layout: [-FRT*R0 | -FRT*t0 | -FRT*R1 | -FRT*t1 | R2 | t2]
        rrep = bass.AP(gt, go + GW, [gpart, [4, 3], [0, J], [1, 4]])

        # ---------- project gaussians ----------
        # pc[p, i, j] = sum_{c<4} gd[p, j*9+c] * rT[4i + c]
        # (gd fields [posx posy posz 1], so the translation rides the reduce)
        aa = pool.tile([P, J], f32, tag="aa")     # 1/scale
        nc.vector.reciprocal_approx_fast(aa[:], scal)
        m_ = pool.tile([P, 3, J, 4], f32, tag="m_")
        pc = pool.tile([P, 3, J], f32, tag="pc")
        posrep = bass.AP(gt, go, [gpart, [0, 3], [9, J], [1, 4]])
        nc.vector.tensor_tensor(m_[:], posrep, rrep, ALU.mult)
        nc.vector.tensor_reduce(pc[:], m_[:], mybir.AxisListType.X, ALU.add)

        invz = pool.tile([P, J], f32, tag="invz")
        nc.vector.reciprocal_approx_fast(invz[:], pc[:, 2, :])
        # rT's x/y rows are pre-scaled by -F*sqrt(.5) and the grids are
        # pre-shifted by -C, so uvt = pc_xy/pcz is directly the negated
        # argbuild bias (times 1/scale).
        uvt = pool.tile([P, 2, J], f32, tag="uvt")
        invzrep = bass.AP(invz[:].tensor, invz[:].offset,
                          [invz[:].ap[0], [0, 2], [1, J]])
        nc.vector.tensor_tensor(uvt[:], pc[:, 0:2, :], invzrep, ALU.mult)

        buv = pool.tile([P, 2, J], f32, tag="buv")  # [u'/scale, v'/scale]
        aarep = bass.AP(aa[:].tensor, aa[:].offset, [aa[:].ap[0], [0, 2], [1, J]])
        nc.vector.tensor_tensor(buv[:], uvt[:], aarep, ALU.mult)
        bu, bv = buv[:, 0, :], buv[:, 1, :]

        # s4[:, k, j]: per-gaussian weights [opac, opac*c] — on gpsimd,
        # which is otherwise idle
        s4b = pool.tile([P, 4, J], bf16, tag="s4b")
        nc.gpsimd.tensor_copy(s4b[:, 0, :], opac)
        for k in range(3):
            nc.gpsimd.tensor_tensor(s4b[:, 1 + k, :], opac, gfield(4 + k), ALU.mult)

        # ---------- pipelined main phase ----------
        # Per segment s: argbuild (y on DVE, x on gpsimd) -> square+exp
        # (ACT) -> rhs-build (DVE) -> accumulate matmuls (PE). rhs(s-1) is
        # emitted after argbuild(s) so the in-order DVE stream never waits
        # on ACT.
        acc = psum_pool.tile([P, 4 * SW], f32, tag="acc")
        expv_tiles = []

        def emit_argbuild(s, j0, seg):
            args = args_pool.tile([P, seg, H + SW], f32, tag=f"args{s}")
            argsq = sq_pool.tile([P, seg, H + SW], f32, tag="argsq")
            expv = exp_pool.tile([P, seg, H + SW], bf16, tag=f"expv{s}")
            expv_tiles.append(expv)
            for jj in range(seg):
                j = j0 + jj
                nc.vector.tensor_scalar(
                    args[:, jj, 0:H], ygrid, aa[:, j:j + 1], bv[:, j:j + 1],
                    ALU.mult, ALU.add,
                )
                nc.gpsimd.tensor_scalar(
                    args[:, jj, H:H + SW], xgrid, aa[:, j:j + 1], bu[:, j:j + 1],
                    ALU.mult, ALU.add,
                )
            if s in DVE_SQ_SEGS:
                # square on the DVE — it is otherwise idle at this point
                nc.vector.tensor_tensor(
                    argsq[:], args[:], args[:], ALU.mult,
                )
            else:
                nc.scalar.activation(argsq[:], args[:], AF.Square)
            nc.scalar.activation(expv[:], argsq[:], AF.Exp, scale=-1.0)

        def emit_consume(s, j0, seg):
            # rhs[:, jj, k*SW + x] = xexp[jj, x] * s4b[k, j0+jj] for the
            # whole segment in ONE op via repeat-read APs (step-0 dims)
            expv = expv_tiles[s]
            rhs = rhs_pool.tile([P, seg, 4 * SW], bf16, tag="rhs")
            part = expv[:].ap[0]
            xrep = bass.AP(
                expv[:].tensor, expv[:, 0, H:H + SW].offset,
                [part, [H + SW, seg], [0, 4], [1, SW]],
            )
            srep = bass.AP(
                s4b[:].tensor, s4b[:, :, j0].offset,
                [s4b[:].ap[0], [1, seg], [J, 4], [0, SW]],
            )
            rout = bass.AP(
                rhs[:].tensor, rhs[:].offset,
                [rhs[:].ap[0], [4 * SW, seg], [SW, 4], [1, SW]],
            )
            nc.vector.tensor_tensor(rout, xrep, srep, ALU.mult)
            for jj in range(seg):
                j = j0 + jj
                nc.tensor.matmul(
                    acc[:], expv[:, jj, 0:H], rhs[:, jj, :],
                    start=(j == 0), stop=False,
                )

        starts = [sum(SEGS[:i]) for i in range(len(SEGS))]
        for s, (j0, seg) in enumerate(zip(starts, SEGS)):
            emit_argbuild(s, j0, seg)
            if s >= 1:
                emit_consume(s - 1, starts[s - 1], SEGS[s - 1])
        emit_consume(len(SEGS) - 1, starts[-1], SEGS[-1])
        # fold den += EPS into the accumulation group (K=1 matmul of
        # ones x eps) so the divide chain needs no separate add
        nc.tensor.matmul(acc[:, 0:SW], ones1[:], epsr[:], start=False, stop=True)

        # ---------- divide and store ----------
        # all four planes are divided ([den*dinv | c*dinv x3]); the host
        # drops plane 0. This keeps the DMA rows at 512B (full-rate bucket).
        dinv = pool.tile([P, SW], f32, tag="dinv")
        nc.vector.reciprocal_approx_fast(dinv[:], acc[:, 0:SW])
        outsb = pool.tile([P, 4, SW], f32, tag="outsb")
        dinv4 = bass.AP(
            dinv[:].tensor, dinv[:].offset, [dinv[:].ap[0], [0, 4], [1, SW]]
        )
        nc.vector.tensor_tensor(outsb[:], acc[:, 0:4 * SW], dinv4, ALU.mult)
        nc.sync.dma_start(aps["out"], outsb[:])


def _emit_empty(nc, tc, aps):
    """J == 0: the culled work list is empty, i.e. no gaussian reaches any
    pixel of this core's strip — every weight is an exact f32 zero, so
    num = 0, den = EPS and the strip renders to exact zeros."""
    with ExitStack() as ctx:
        pool = ctx.enter_context(tc.tile_pool(name="main", bufs=1))
        outsb = pool.tile([P, 4, SW], f32, tag="outsb")
        nc.gpsimd.memset(outsb[:], 0.0)
        nc.sync.dma_start(aps["out"], outsb[:])


def build_nc(J, num_devices=8, debug=False):
    nc = bacc.Bacc(
        "TRN2", target_bir_lowering=False, debug=debug, num_devices=num_devices
    )
    # out[y, k, x]: partition-major like the SBUF result tile (plane 0 is
    # den*dinv padding); the host keeps planes 1:4 and transposes back to
    # [c, y, x]
    aps = {"out": nc.dram_tensor("out", [H, 4, SW], f32, kind="ExternalOutput").ap()}
    if J > 0:
        aps["gdata"] = nc.dram_tensor(
            "gdata", [P, J * 9 + 12], f32, kind="ExternalInput").ap()
        aps["smalls"] = nc.dram_tensor(
            "smalls", [H + SW], f32, kind="ExternalInput").ap()
    with tile.TileContext(nc) as tc:
        if J > 0:
            _emit(nc, tc, aps, J)
        else:
            _emit_empty(nc, tc, aps)
    nc.compile()
    return nc


def _rot_from_quat(q):
    q = np.asarray(q, np.float64)
    q = q / np.linalg.norm(q)
    w, x, y, z = q
    return np.array([
        [1 - 2 * (y * y + z * z), 2 * (x * y - z * w), 2 * (x * z + y * w)],
        [2 * (x * y + z * w), 1 - 2 * (x * x + z * z), 2 * (y * z - x * w)],
        [2 * (x * z - y * w), 2 * (y * z + x * w), 1 - 2 * (x * x + y * y)],
    ])


def plan(positions, colors, opacities, scales, qvec, tvec):
    """Cull per (batch, strip), choose J = max chunk count."""
    pos = np.asarray(positions, np.float64)
    scal = np.asarray(scales, np.float64).reshape(-1)
    var = scal * scal
    keeps = []
    for b in range(B):
        R = _rot_from_quat(qvec[b])
        p_cam = pos @ R.T + np.asarray(tvec[b], np.float64)
        with np.errstate(divide="ignore", invalid="ignore"):
            u = p_cam[:, 0] / p_cam[:, 2] * FX + CX
            v = p_cam[:, 1] / p_cam[:, 2] * FY + CY
        dy = np.maximum(np.maximum(0.0 - v, v - (H - 1)), 0.0)
        dy2 = dy * dy
        for strip in range(NSTRIP):
            x0, x1 = strip * SW, strip * SW + SW - 1
            dx = np.maximum(np.maximum(x0 - u, u - x1), 0.0)
            d2 = dx * dx + dy2
            keeps.append(np.nonzero(d2 <= CULL_THRESH * var)[0])
    J = (max(len(k) for k in keeps) + P - 1) // P
    return keeps, J


def make_in_maps(keeps, J, positions, colors, opacities, scales, qvec, tvec):
    if J == 0:
        return [{} for _ in range(8)]
    # grids pre-scaled by sqrt(.5) and pre-shifted by the principal point;
    # rT's x/y rows carry -F*sqrt(.5) so that the device's uvt = pc_xy/pcz
    # is directly the negated argbuild bias (times 1/scale)
    ygrid = ((np.arange(H, dtype=np.float64) - CY) * RT).astype(np.float32)
    gall = np.empty((N, 9), np.float32)
    gall[:, 0:3] = np.asarray(positions, np.float32)
    gall[:, 3] = 1.0
    gall[:, 4:7] = np.asarray(colors, np.float32)
    gall[:, 7] = np.asarray(opacities, np.float32).reshape(N)
    gall[:, 8] = np.asarray(scales, np.float32).reshape(N)
    # padding gaussian: in front of the camera, zero opacity
    pad = np.array([0, 0, 1, 1, 0, 0, 0, 0, 1], np.float32)
    in_maps = []
    for core in range(8):
        b, strip = core // NSTRIP, core % NSTRIP
        keep = keeps[core]
        g = np.empty((J * P, 9), np.float32)
        g[:len(keep)] = gall[keep]
        g[len(keep):] = pad
        R = _rot_from_quat(qvec[b])
        t = np.asarray(tvec[b], np.float64)
        rT = np.empty((3, 4), np.float64)
        rT[:, 0:3] = R
        rT[:, 3] = t
        rT[0:2] *= -FX * RT
        # gdata[p, :] = [chunk0 fields | chunk1 fields | ... | rT]
        gdata = np.empty((P, J * 9 + 12), np.float32)
        gdata[:, :J * 9] = g.reshape(J, P, 9).transpose(1, 0, 2).reshape(P, J * 9)
        gdata[:, J * 9:] = rT.reshape(-1).astype(np.float32)[None, :]
        xgrid = ((np.arange(SW, dtype=np.float64) + strip * SW - CX) * RT
                 ).astype(np.float32)
        smalls = np.concatenate([ygrid, xgrid]).astype(np.float32)
        in_maps.append({"gdata": gdata, "smalls": smalls})
    return in_maps


_NC_CACHE = {}


def _get_nc(J):
    if J not in _NC_CACHE:
        _NC_CACHE[J] = build_nc(J)
    return _NC_CACHE[J]


def unshard(results):
    """results: per-core 'out' [H, 4, SW] -> full [B, 3, H, W]."""
    out = np.empty((B, 3, H, W), np.float32)
    for core in range(8):
        b, strip = core // NSTRIP, core % NSTRIP
        out[b, :, :, strip * SW:(strip + 1) * SW] = (
            np.asarray(results[core]["out"])[:, 1:4, :].transpose(1, 0, 2)
        )
    return out


def run_spmd(inputs, trace=False, **kw):
    keeps, J = plan(**inputs)
    nc = _get_nc(J)
    in_maps = make_in_maps(keeps, J, **inputs)
    try:
        res = run_bass_kernel_spmd(nc, in_maps, list(range(8)), trace=trace, **kw)
    except Exception:
        # transient device errors (e.g. NRT_EXEC_UNIT_UNRECOVERABLE) have
        # been observed to clear on retry
        res = run_bass_kernel_spmd(nc, in_maps, list(range(8)), trace=trace, **kw)
    return unshard(res.results), res


def kernel(positions, colors, opacities, scales, qvec, tvec):
    out, _ = run_spmd(dict(
        positions=positions, colors=colors, opacities=opacities,
        scales=scales, qvec=qvec, tvec=tvec,
    ))
    return out


# revision 10
# speedup vs baseline: 6.0503x; 4.7208x over previous
"""Trainium2 Bass kernel for the differentiable gaussian renderer.

Math: for each batch b, pixel (y,x), channel c:
    out[b,c,y,x] = num/den,
    num = sum_n colors[n,c] * w[n,y,x],  den = eps + sum_n w[n,y,x]
    w[n,y,x] = opac_n * exp(-0.5*((x-u_n)^2 + (y-v_n)^2)/s_n^2)

Key restructurings:
  * w is separable over the pixel grid, so we only need exp on [N,128]
    y-factors and [N,32] x-factors (64x fewer exps than the dense [N,HW]
    formulation), and num/den become a single accumulating bf16 matmul
    over N on the tensor engine:
        psum[y, (k,x)] = sum_n Yexp[n,y] * (s4[n,k] * Xexp[n,x])
    where s4 = [opac, opac*c0, opac*c1, opac*c2] (k=0 gives den).
  * tile culling (standard splatting): a gaussian whose nearest pixel in
    a core's strip is further than sqrt(300)*scale contributes
    exp(-0.5*d2/var) with 0.5*d2/var > 150, which underflows to an exact
    float32 zero — dropping it leaves the f32 result bit-unchanged. Each
    core receives only the gaussians that can touch its strip; the device
    program is compiled for the runtime maximum chunk count J.

Sharding: 8 cores = 2 batches x 4 pixel-column strips of 32. No
collectives; the host just concatenates the per-core strip outputs.
"""

import math
from contextlib import ExitStack

import numpy as np

import concourse.bacc as bacc
import concourse.bass as bass
import concourse.mybir as mybir
import concourse.tile as tile
from concourse.bass_utils import run_bass_kernel_spmd

f32 = mybir.dt.float32
bf16 = mybir.dt.bfloat16

H, W = 128, 128
FX, FY = 150.0, 150.0
CX, CY = 64.0, 64.0
EPS = 1e-8
N, B = 4096, 2
P = 128          # SBUF partitions; gaussians per contraction chunk
NSTRIP = 4       # x strips
SW = W // NSTRIP  # 32 columns per strip
RT = math.sqrt(0.5)
# cull threshold: drop when d2 > CULL_THRESH * var, i.e. 0.5*d2/var > 150;
# exp(-150) = 7e-66 is far below the smallest f32 denormal, so dropped
# gaussians contribute exact zeros in the reference's own f32 arithmetic.
CULL_THRESH = 300.0

AF = mybir.ActivationFunctionType
ALU = mybir.AluOpType


def _segs_for(J):
    """Pipeline segment sizes: small first segment so ACT starts early,
    size-1 last segment so the post-exp tail is short."""
    if J <= 2:
        return [J]
    segs = [min(3, J - 1)]
    rem = J - segs[0] - 1
    while rem > 0:
        take = min(8, rem)
        segs.append(take)
        rem -= take
    segs.append(1)
    return segs


def _emit(nc, tc, aps, J):
    SEGS = _segs_for(J)
    GW = J * 9  # flat width of the per-gaussian block in the packed tile

    with ExitStack() as ctx:
        pool = ctx.enter_context(tc.tile_pool(name="main", bufs=1))
        args_pool = ctx.enter_context(tc.tile_pool(name="args", bufs=1))
        exp_pool = ctx.enter_context(tc.tile_pool(name="exp", bufs=1))
        rhs_pool = ctx.enter_context(tc.tile_pool(name="rhs", bufs=3))
        psum_pool = ctx.enter_context(tc.tile_pool(name="psum", bufs=1, space="PSUM"))

        # ---------- load inputs ----------
        # gdata[p, j*9:(j+1)*9] = [posx posy posz 1 c0 c1 c2 opac scal] of
        # the p-th gaussian of chunk j (host pre-packs after culling), and
        # gdata[p, J*9:J*9+12] = rT (host-replicated): the whole serial
        # projection chain depends on this single DMA, which goes FIRST on
        # the SP/HWDGE queue. The pixel grids ride the gpsimd SWDGE queue
        # in parallel — they are only needed later, at argbuild.
        gd = pool.tile([P, GW + 12], f32, tag="gd")
        sm = pool.tile([P, H + SW], f32, tag="sm")
        nc.sync.dma_start(gd[:], aps["gdata"])
        nc.gpsimd.dma_start(sm[:], aps["smalls"].partition_broadcast(P))
        # prime the ACT table set (erf_derivative) while waiting on the DMAs.
        # Derivative_Erf(x) = 2/sqrt(pi) * exp(-x^2) is the separable
        # gaussian factor itself — the host folds (sqrt(pi)/2)^2 into s4.
        warm = pool.tile([P, 1], f32, tag="warm")
        nc.gpsimd.memset(warm[:], 0.0)
        nc.scalar.activation(warm[:], warm[:], AF.Derivative_Erf)
        # constants for the eps-fold matmul at the end of the accumulation
        ones1 = pool.tile([1, P], f32, tag="ones1")
        epsr = pool.tile([1, SW], f32, tag="epsr")
        nc.gpsimd.memset(ones1[:], 1.0)
        nc.gpsimd.memset(epsr[:], EPS)
        ygrid = sm[:, 0:H]
        xgrid = sm[:, H:H + SW]

        gt, go = gd[:].tensor, gd[:].offset
        gpart = gd[:].ap[0]

        def gfield(c):  # [P, J] view of per-gaussian field c
            return bass.AP(gt, go + c, [gpart, [9, J]])

        opac, scal = gfield(7), gfield(8)
        # rT layout: [-FRT*R0 | -FRT*t0 | -FRT*R1 | -FRT*t1 | R2 | t2]
        rrep = bass.AP(gt, go + GW, [gpart, [4, 3], [0, J], [1, 4]])

        # ---------- project gaussians ----------
        # pc[p, i, j] = sum_{c<4} gd[p, j*9+c] * rT[4i + c]
        # (gd fields [posx posy posz 1], so the translation rides the reduce)
        aa = pool.tile([P, J], f32, tag="aa")     # 1/scale
        nc.vector.reciprocal_approx_fast(aa[:], scal)
        m_ = pool.tile([P, 3, J, 4], f32, tag="m_")
        pc = pool.tile([P, 3, J], f32, tag="pc")
        posrep = bass.AP(gt, go, [gpart, [0, 3], [9, J], [1, 4]])
        nc.vector.tensor_tensor(m_[:], posrep, rrep, ALU.mult)
        nc.vector.tensor_reduce(pc[:], m_[:], mybir.AxisListType.X, ALU.add)

        invz = pool.tile([P, J], f32, tag="invz")
        nc.vector.reciprocal_approx_fast(invz[:], pc[:, 2, :])
        # rT's x/y rows are pre-scaled by -F*sqrt(.5) and the grids are
        # pre-shifted by -C, so uvt = pc_xy/pcz is directly the negated
        # argbuild bias (times 1/scale).
        uvt = pool.tile([P, 2, J], f32, tag="uvt")
        invzrep = bass.AP(invz[:].tensor, invz[:].offset,
                          [invz[:].ap[0], [0, 2], [1, J]])
        nc.vector.tensor_tensor(uvt[:], pc[:, 0:2, :], invzrep, ALU.mult)

        buv = pool.tile([P, 2, J], f32, tag="buv")  # [u'/scale, v'/scale]
        aarep = bass.AP(aa[:].tensor, aa[:].offset, [aa[:].ap[0], [0, 2], [1, J]])
        nc.vector.tensor_tensor(buv[:], uvt[:], aarep, ALU.mult)
        bu, bv = buv[:, 0, :], buv[:, 1, :]

        # s4[:, k, j]: per-gaussian weights [opac, opac*c] — on gpsimd,
        # which is otherwise idle
        s4b = pool.tile([P, 4, J], bf16, tag="s4b")
        nc.gpsimd.tensor_copy(s4b[:, 0, :], opac)
        for k in range(3):
            nc.gpsimd.tensor_tensor(s4b[:, 1 + k, :], opac, gfield(4 + k), ALU.mult)

        # ---------- pipelined main phase ----------
        # Per segment s: argbuild (y on DVE, x on gpsimd) -> square+exp
        # (ACT) -> rhs-build (DVE) -> accumulate matmuls (PE). rhs(s-1) is
        # emitted after argbuild(s) so the in-order DVE stream never waits
        # on ACT.
        acc = psum_pool.tile([P, 4 * SW], f32, tag="acc")
        expv_tiles = []

        def emit_argbuild(s, j0, seg):
            args = args_pool.tile([P, seg, H + SW], f32, tag=f"args{s}")
            expv = exp_pool.tile([P, seg, H + SW], bf16, tag=f"expv{s}")
            expv_tiles.append(expv)
            for jj in range(seg):
                j = j0 + jj
                nc.vector.tensor_scalar(
                    args[:, jj, 0:H], ygrid, aa[:, j:j + 1], bv[:, j:j + 1],
                    ALU.mult, ALU.add,
                )
                nc.gpsimd.tensor_scalar(
                    args[:, jj, H:H + SW], xgrid, aa[:, j:j + 1], bu[:, j:j + 1],
                    ALU.mult, ALU.add,
                )
            # fused square+exp: Derivative_Erf(t) = 2/sqrt(pi) * exp(-t^2)
            nc.scalar.activation(expv[:], args[:], AF.Derivative_Erf)

        def emit_consume(s, j0, seg):
            # rhs[:, jj, k*SW + x] = xexp[jj, x] * s4b[k, j0+jj] via
            # repeat-read APs (step-0 dims), split between the DVE and the
            # otherwise-idle gpsimd
            expv = expv_tiles[s]
            rhs = rhs_pool.tile([P, seg, 4 * SW], bf16, tag="rhs")
            part = expv[:].ap[0]

            def rhs_tt(eng, a, b):
                n = b - a
                xrep = bass.AP(
                    expv[:].tensor, expv[:, a, H:H + SW].offset,
                    [part, [H + SW, n], [0, 4], [1, SW]],
                )
                srep = bass.AP(
                    s4b[:].tensor, s4b[:, :, j0 + a].offset,
                    [s4b[:].ap[0], [1, n], [J, 4], [0, SW]],
                )
                rout = bass.AP(
                    rhs[:].tensor, rhs[:, a, :].offset,
                    [rhs[:].ap[0], [4 * SW, n], [SW, 4], [1, SW]],
                )
                eng.tensor_tensor(rout, xrep, srep, ALU.mult)

            # gpsimd is ~2x slower per element here, so give it the tail
            # third of each segment
            split = max(0, seg - max(1, (seg + 1) // 3))
            if split > 0:
                rhs_tt(nc.vector, 0, split)
            if split < seg:
                rhs_tt(nc.gpsimd, split, seg)
            for jj in range(seg):
                j = j0 + jj
                nc.tensor.matmul(
                    acc[:], expv[:, jj, 0:H], rhs[:, jj, :],
                    start=(j == 0), stop=False,
                )

        starts = [sum(SEGS[:i]) for i in range(len(SEGS))]
        for s, (j0, seg) in enumerate(zip(starts, SEGS)):
            emit_argbuild(s, j0, seg)
            if s >= 1:
                emit_consume(s - 1, starts[s - 1], SEGS[s - 1])
        emit_consume(len(SEGS) - 1, starts[-1], SEGS[-1])
        # fold den += EPS into the accumulation group (K=1 matmul of
        # ones x eps) so the divide chain needs no separate add
        nc.tensor.matmul(acc[:, 0:SW], ones1[:], epsr[:], start=False, stop=True)

        # ---------- divide and store ----------
        # all four planes are divided ([den*dinv | c*dinv x3]); the host
        # drops plane 0. This keeps the DMA rows at 512B (full-rate bucket).
        dinv = pool.tile([P, SW], f32, tag="dinv")
        nc.vector.reciprocal_approx_fast(dinv[:], acc[:, 0:SW])
        outsb = pool.tile([P, 4, SW], f32, tag="outsb")
        dinv4 = bass.AP(
            dinv[:].tensor, dinv[:].offset, [dinv[:].ap[0], [0, 4], [1, SW]]
        )
        nc.vector.tensor_tensor(outsb[:], acc[:, 0:4 * SW], dinv4, ALU.mult)
        nc.sync.dma_start(aps["out"], outsb[:])


def _emit_empty(nc, tc, aps):
    """J == 0: the culled work list is empty, i.e. no gaussian reaches any
    pixel of this core's strip — every weight is an exact f32 zero, so
    num = 0, den = EPS and the strip renders to exact zeros."""
    with ExitStack() as ctx:
        pool = ctx.enter_context(tc.tile_pool(name="main", bufs=1))
        outsb = pool.tile([P, 4, SW], f32, tag="outsb")
        nc.gpsimd.memset(outsb[:], 0.0)
        nc.sync.dma_start(aps["out"], outsb[:])


def build_nc(J, num_devices=8, debug=False):
    nc = bacc.Bacc(
        "TRN2", target_bir_lowering=False, debug=debug, num_devices=num_devices
    )
    # out[y, k, x]: partition-major like the SBUF result tile (plane 0 is
    # den*dinv padding); the host keeps planes 1:4 and transposes back to
    # [c, y, x]
    aps = {"out": nc.dram_tensor("out", [H, 4, SW], f32, kind="ExternalOutput").ap()}
    if J > 0:
        aps["gdata"] = nc.dram_tensor(
            "gdata", [P, J * 9 + 12], f32, kind="ExternalInput").ap()
        aps["smalls"] = nc.dram_tensor(
            "smalls", [H + SW], f32, kind="ExternalInput").ap()
    with tile.TileContext(nc) as tc:
        if J > 0:
            _emit(nc, tc, aps, J)
        else:
            _emit_empty(nc, tc, aps)
    nc.compile()
    return nc


def _rot_from_quat(q):
    q = np.asarray(q, np.float64)
    q = q / np.linalg.norm(q)
    w, x, y, z = q
    return np.array([
        [1 - 2 * (y * y + z * z), 2 * (x * y - z * w), 2 * (x * z + y * w)],
        [2 * (x * y + z * w), 1 - 2 * (x * x + z * z), 2 * (y * z - x * w)],
        [2 * (x * z - y * w), 2 * (y * z + x * w), 1 - 2 * (x * x + y * y)],
    ])


def plan(positions, colors, opacities, scales, qvec, tvec):
    """Cull per (batch, strip), choose J = max chunk count."""
    pos = np.asarray(positions, np.float64)
    scal = np.asarray(scales, np.float64).reshape(-1)
    var = scal * scal
    keeps = []
    for b in range(B):
        R = _rot_from_quat(qvec[b])
        p_cam = pos @ R.T + np.asarray(tvec[b], np.float64)
        with np.errstate(divide="ignore", invalid="ignore"):
            u = p_cam[:, 0] / p_cam[:, 2] * FX + CX
            v = p_cam[:, 1] / p_cam[:, 2] * FY + CY
        dy = np.maximum(np.maximum(0.0 - v, v - (H - 1)), 0.0)
        dy2 = dy * dy
        for strip in range(NSTRIP):
            x0, x1 = strip * SW, strip * SW + SW - 1
            dx = np.maximum(np.maximum(x0 - u, u - x1), 0.0)
            d2 = dx * dx + dy2
            keeps.append(np.nonzero(d2 <= CULL_THRESH * var)[0])
    J = (max(len(k) for k in keeps) + P - 1) // P
    return keeps, J


def make_in_maps(keeps, J, positions, colors, opacities, scales, qvec, tvec):
    if J == 0:
        return [{} for _ in range(8)]
    # grids pre-scaled by sqrt(.5) and pre-shifted by the principal point;
    # rT's x/y rows carry -F*sqrt(.5) so that the device's uvt = pc_xy/pcz
    # is directly the negated argbuild bias (times 1/scale)
    ygrid = ((np.arange(H, dtype=np.float64) - CY) * RT).astype(np.float32)
    gall = np.empty((N, 9), np.float32)
    gall[:, 0:3] = np.asarray(positions, np.float32)
    gall[:, 3] = 1.0
    gall[:, 4:7] = np.asarray(colors, np.float32)
    # fold (sqrt(pi)/2)^2 into the opacity: the device's separable factors
    # come from Derivative_Erf(t) = 2/sqrt(pi) * exp(-t^2)
    gall[:, 7] = (np.asarray(opacities, np.float64).reshape(N) *
                  (math.pi / 4.0)).astype(np.float32)
    gall[:, 8] = np.asarray(scales, np.float32).reshape(N)
    # padding gaussian: in front of the camera, zero opacity
    pad = np.array([0, 0, 1, 1, 0, 0, 0, 0, 1], np.float32)
    in_maps = []
    for core in range(8):
        b, strip = core // NSTRIP, core % NSTRIP
        keep = keeps[core]
        g = np.empty((J * P, 9), np.float32)
        g[:len(keep)] = gall[keep]
        g[len(keep):] = pad
        R = _rot_from_quat(qvec[b])
        t = np.asarray(tvec[b], np.float64)
        rT = np.empty((3, 4), np.float64)
        rT[:, 0:3] = R
        rT[:, 3] = t
        rT[0:2] *= -FX * RT
        # gdata[p, :] = [chunk0 fields | chunk1 fields | ... | rT]
        gdata = np.empty((P, J * 9 + 12), np.float32)
        gdata[:, :J * 9] = g.reshape(J, P, 9).transpose(1, 0, 2).reshape(P, J * 9)
        gdata[:, J * 9:] = rT.reshape(-1).astype(np.float32)[None, :]
        xgrid = ((np.arange(SW, dtype=np.float64) + strip * SW - CX) * RT
                 ).astype(np.float32)
        smalls = np.concatenate([ygrid, xgrid]).astype(np.float32)
        in_maps.append({"gdata": gdata, "smalls": smalls})
    return in_maps


_NC_CACHE = {}


def _get_nc(J):
    if J not in _NC_CACHE:
        _NC_CACHE[J] = build_nc(J)
    return _NC_CACHE[J]


def unshard(results):
    """results: per-core 'out' [H, 4, SW] -> full [B, 3, H, W]."""
    out = np.empty((B, 3, H, W), np.float32)
    for core in range(8):
        b, strip = core // NSTRIP, core % NSTRIP
        out[b, :, :, strip * SW:(strip + 1) * SW] = (
            np.asarray(results[core]["out"])[:, 1:4, :].transpose(1, 0, 2)
        )
    return out


def run_spmd(inputs, trace=False, **kw):
    keeps, J = plan(**inputs)
    nc = _get_nc(J)
    in_maps = make_in_maps(keeps, J, **inputs)
    try:
        res = run_bass_kernel_spmd(nc, in_maps, list(range(8)), trace=trace, **kw)
    except Exception:
        # transient device errors (e.g. NRT_EXEC_UNIT_UNRECOVERABLE) have
        # been observed to clear on retry
        res = run_bass_kernel_spmd(nc, in_maps, list(range(8)), trace=trace, **kw)
    return unshard(res.results), res


def kernel(positions, colors, opacities, scales, qvec, tvec):
    out, _ = run_spmd(dict(
        positions=positions, colors=colors, opacities=opacities,
        scales=scales, qvec=qvec, tvec=tvec,
    ))
    return out
